# revision 11
# baseline (speedup 1.0000x reference)
"""Multi-head attention Trainium2 kernel (8 NeuronCores), v2.

Sharding: core c = 4*b + l handles batch b and head group l (4 of 16 heads,
as 2 head-pairs p=0,1), like v1.  New in v2: the q-row blocks are processed
in XOR-rotated order [l^1, l^2, l^3, l] (host packs xq per-core in that
slot order; program is identical SPMD).  After attention slot j<3 the core
has exactly the 512 q rows owned by peer l^(j+1) — exchanged via a tiny
2-rank AllGather (256 KB) that fully overlaps the next ~30us attention
slot.  The own block is processed LAST and needs no communication, so the
tail is just the local output projection: no exposed collective (v1 idled
the PE ~43us waiting for its second AllToAll).

Attention math per (slot, pair) unit is v1's: scoresT via lhsT=kT-chunk,
rhs=qT (softmax axis = partition); exp fused on ScalarE (scale=1/8);
rowsum via 64 ones columns packed next to v; oT normalized with
reciprocal+shift before the exchange.
"""

import sys

if "/opt/trn_rl_repo" not in sys.path:
    sys.path.insert(0, "/opt/trn_rl_repo")

import numpy as np
import ml_dtypes

import concourse.bass as bass  # noqa: F401
import concourse.bacc as bacc
import concourse.bass_utils as bass_utils
import concourse.mybir as mybir
import concourse.tile as tile

B, S, DIN = 2, 2048, 1024
H, DK = 16, 64
NCORES = 8
HL = 4  # heads per core
SQ = S // 4  # output rows per core
DC = DIN // 128  # 8 din chunks
SKC = S // 128  # 16 s_k chunks
VW = 2 * DK  # 128: 64 v columns + 64 ones columns (rowsum via PE)

F32 = mybir.dt.float32
BF16 = mybir.dt.bfloat16
BF = ml_dtypes.bfloat16

# round j exchanges with peer l ^ DELTAS[j]; 2-rank groups, same-batch
# quads.  Adjacent pairs (delta 1) get the fast MESH collective algorithm,
# strided pairs get a slow RING — so the adjacent exchange goes LAST where
# the collective is tail-critical, the ring rounds run early with slack.
DELTAS = [2, 3, 1]
RG = [
    [[0, 2], [1, 3], [4, 6], [5, 7]],  # delta 2 (ring, lots of slack)
    [[0, 3], [1, 2], [4, 7], [5, 6]],  # delta 3 (ring, slack)
    [[0, 1], [2, 3], [4, 5], [6, 7]],  # delta 1 (mesh, tail-critical)
]


def build(dbg=False):
    nc = bacc.Bacc("TRN2", target_bir_lowering=False, debug=False, num_devices=NCORES)

    # ---- DRAM tensors (all host-prearranged, see make_in_maps) ----
    xq = nc.dram_tensor("xq", [128, 4 * DC * 512], BF16, kind="ExternalInput")
    xk = nc.dram_tensor("xk", [128, 4 * DC * 512], BF16, kind="ExternalInput")
    xv = nc.dram_tensor("xv", [128, 4 * DC * 512], BF16, kind="ExternalInput")
    wq = nc.dram_tensor("wq", [128, DC * 256], BF16, kind="ExternalInput")
    wk = nc.dram_tensor("wk", [128, DC * 256], BF16, kind="ExternalInput")
    wv = nc.dram_tensor("wv", [128, DC * 256], BF16, kind="ExternalInput")
    wo = nc.dram_tensor("wo", [128, DC * DIN], BF16, kind="ExternalInput")
    bqp = nc.dram_tensor("bqp", [128, 2], F32, kind="ExternalInput")
    bkp = nc.dram_tensor("bkp", [128, 2], F32, kind="ExternalInput")
    bvr = nc.dram_tensor("bvr", [128, HL * DK], F32, kind="ExternalInput")
    bor = nc.dram_tensor("bor", [128, DIN], F32, kind="ExternalInput")
    out = nc.dram_tensor("out", [SQ, DIN], F32, kind="ExternalOutput")

    cc_in = [
        nc.dram_tensor(f"cc_in{j}", [2 * 128, SQ], BF16, kind="Internal")
        for j in range(3)
    ]
    cc_out = [
        nc.dram_tensor(f"cc_out{j}", [4 * 128, SQ], BF16, kind="Internal")
        for j in range(3)
    ]
    if dbg:
        d_qt = nc.dram_tensor("d_qt", [128, 2 * S], BF16, kind="ExternalOutput")
        d_kt = nc.dram_tensor("d_kt", [128, 2 * S], BF16, kind="ExternalOutput")
        d_ol = nc.dram_tensor("d_ol", [128, 8 * 512], BF16, kind="ExternalOutput")

    with tile.TileContext(nc) as tc:
        with (
            tc.tile_pool(name="pers", bufs=1) as pers,
            tc.tile_pool(name="work", bufs=3) as work,
            tc.tile_pool(name="wrk2", bufs=2) as wrk2,
            tc.tile_pool(name="recv", bufs=1) as recv,
            tc.tile_pool(name="psmm", bufs=2, space="PSUM") as psmm,
            tc.tile_pool(name="psacc", bufs=4, space="PSUM") as psacc,
        ):
            # ---- small per-partition constants (sync queue) ----
            bq_sb = pers.tile([128, 2], F32)
            bk_sb = pers.tile([128, 2], F32)
            bv_sb = pers.tile([128, HL, DK], F32)
            bo_sb = pers.tile([128, DIN], F32)
            # ---- weights (scalar HWDGE queue, starts immediately) ----
            wq_sb = pers.tile([128, DC, 256], BF16)
            wk_sb = pers.tile([128, DC, 256], BF16)
            wv_sb = pers.tile([128, DC, 256], BF16)
            nc.scalar.dma_start(wq_sb[:], wq.rearrange("p (c d) -> p c d", c=DC))
            nc.scalar.dma_start(wk_sb[:], wk.rearrange("p (c d) -> p c d", c=DC))
            nc.scalar.dma_start(wv_sb[:], wv.rearrange("p (c d) -> p c d", c=DC))
            nc.sync.dma_start(bq_sb[:], bqp[:])
            nc.sync.dma_start(bk_sb[:], bkp[:])
            nc.sync.dma_start(bv_sb[:], bvr.rearrange("p (h d) -> p h d", h=HL))
            nc.sync.dma_start(bo_sb[:], bor[:])

            # ---- X loads, s-block-major, spread over four HWDGE queues so
            # the exp-paced attention start isn't DMA-starved ----
            xq_sb = pers.tile([128, 4, DC, 512], BF16, name="xq_sb")
            xk_sb = pers.tile([128, 4, DC, 512], BF16, name="xk_sb")
            xv_sb = pers.tile([128, 4, DC, 512], BF16, name="xv_sb")

            def load_x(eng, xsb, xdram, sblk):
                eng.dma_start(
                    xsb[:, sblk, :, :],
                    xdram[:, 4096 * sblk : 4096 * (sblk + 1)].rearrange(
                        "p (c s) -> p c s", c=DC
                    ),
                )

            # gpsimd is the only fast bulk queue: x blocks there in
            # consumption order (sync is slow, scalar carries the weights).
            # xv block 0 is split: half rides scalar right after the weights,
            # pulling xv0 AND everything behind it on gpsimd ~3us earlier.
            load_x(nc.gpsimd, xq_sb, xq, 0)
            load_x(nc.gpsimd, xk_sb, xk, 0)
            nc.gpsimd.dma_start(
                xv_sb[:, 0, 0:4, :],
                xv[:, 0:2048].rearrange("p (c s) -> p c s", c=4),
            )
            nc.scalar.dma_start(
                xv_sb[:, 0, 4:8, :],
                xv[:, 2048:4096].rearrange("p (c s) -> p c s", c=4),
            )
            # xk block 1 likewise split so scores iter 4 isn't starved
            nc.gpsimd.dma_start(
                xk_sb[:, 1, 0:4, :],
                xk[:, 4096:6144].rearrange("p (c s) -> p c s", c=4),
            )
            nc.scalar.dma_start(
                xk_sb[:, 1, 4:8, :],
                xk[:, 6144:8192].rearrange("p (c s) -> p c s", c=4),
            )
            for xsb, xdram, sblk in (
                (xk_sb, xk, 2),
                (xv_sb, xv, 1),
                (xk_sb, xk, 3),
                (xv_sb, xv, 2),
                (xv_sb, xv, 3),
                (xq_sb, xq, 1),
                (xq_sb, xq, 2),
                (xq_sb, xq, 3),
            ):
                load_x(nc.gpsimd, xsb, xdram, sblk)
            wo_sb = pers.tile([128, DC, DIN], BF16, name="wo_sb")
            nc.scalar.dma_start(wo_sb[:], wo.rearrange("p (c d) -> p c d", c=DC))

            # ---- v ones columns (rowsum trick); after the vector-queue
            # dma dispatches so it doesn't delay them ----
            v_sb = pers.tile([128, SKC, HL, VW], BF16)
            nc.vector.memset(v_sb[:, :, :, DK:VW], 1.0)

            # ---- projections (emitted piecewise, interleaved with attention)
            qt_sb = [pers.tile([128, S], BF16, name=f"qt{p}") for p in range(2)]
            kt_sb = [pers.tile([128, S], BF16, name=f"kt{p}") for p in range(2)]

            def emit_q(j, p):
                # qT for slot j, pair p: [128, 512]
                ps = psacc.tile([128, 512], F32, tag="acc", name="psq")
                for c in range(DC):
                    nc.tensor.matmul(
                        ps[:],
                        wq_sb[:, c, 128 * p : 128 * (p + 1)],
                        xq_sb[:, j, c, :],
                        start=(c == 0),
                        stop=(c == DC - 1),
                    )
                nc.vector.tensor_scalar_add(
                    qt_sb[p][:, 512 * j : 512 * (j + 1)], ps[:], bq_sb[:, p : p + 1]
                )

            def emit_k(sb, p):
                # kT for k-block sb, pair p: [128, 512]
                ps = psacc.tile([128, 512], F32, tag="acc", name="psk")
                for c in range(DC):
                    nc.tensor.matmul(
                        ps[:],
                        wk_sb[:, c, 128 * p : 128 * (p + 1)],
                        xk_sb[:, sb, c, :],
                        start=(c == 0),
                        stop=(c == DC - 1),
                    )
                nc.vector.tensor_scalar_add(
                    kt_sb[p][:, 512 * sb : 512 * (sb + 1)], ps[:], bk_sb[:, p : p + 1]
                )

            def emit_v(scs):
                for sc in scs:
                    psv = psacc.tile([128, HL, DK], F32, tag="acc", name="psv")
                    for c in range(DC):
                        nc.tensor.matmul(
                            psv[:],
                            xv_sb[:, sc // 4, c, 128 * (sc % 4) : 128 * (sc % 4 + 1)],
                            wv_sb[:, c, :],
                            start=(c == 0),
                            stop=(c == DC - 1),
                        )
                    nc.vector.tensor_add(v_sb[:, sc, :, 0:DK], psv[:], bv_sb[:])

            # ol: the 8 oT chunks for this core's own 512 rows, one tile per
            # chunk so the output projection's dependency is per-chunk
            #   chunks 0..1 = own pairs (slot 3); 2+2j+p = round-j peer pair p
            ol = [pers.tile([128, 512], BF16, name=f"ol{c}") for c in range(8)]
            # send staging, one per round: kept alive because the receive
            # path reconstructs peer = (low + high) - own_send
            snd_t = [
                pers.tile([128, 2, 512], BF16, name=f"snd{j}") for j in range(3)
            ]

            # ---- attention for one (slot, pair) unit + normalize ----
            def emit_attention(j, p, interleave=()):
                """interleave: list of (iter_idx, thunk) emitted before that
                scores iteration — projection work for later slots rides the
                exp-paced inner loop."""
                il = list(interleave)
                qsl = slice(512 * j, 512 * (j + 1))
                po = [
                    psacc.tile([128, 512], F32, tag="acc", name=f"po{ch}")
                    for ch in range(2)
                ]
                ets = [None, None, None]

                def attnv(skc):
                    for ch in range(2):
                        nc.tensor.matmul(
                            po[ch][:],
                            v_sb[:, skc, 2 * p + ch, :],
                            ets[skc % 3][:, 512 * ch : 512 * (ch + 1)],
                            start=(skc == 0),
                            stop=(skc == SKC - 1),
                        )

                for skc in range(SKC):
                    ps2 = psmm.tile([128, 1024], F32, tag="mm", name="ps2")
                    for ch in range(2):
                        cs = slice(64 * ch, 64 * (ch + 1))
                        nc.tensor.matmul(
                            ps2[:, 512 * ch : 512 * (ch + 1)],
                            kt_sb[p][cs, 128 * skc : 128 * (skc + 1)],
                            qt_sb[p][cs, qsl],
                            start=True,
                            stop=True,
                        )
                    et = work.tile([128, 1024], BF16, tag="et", name="et")
                    ets[skc % 3] = et
                    nc.scalar.activation(
                        et[:],
                        ps2[:],
                        mybir.ActivationFunctionType.Exp,
                        bias=0.0,
                        scale=float(1.0 / np.sqrt(DK)),
                    )
                    # interleaved projection work AFTER this iteration's
                    # scores so the exp drumbeat is never pushed back
                    while il and il[0][0] <= skc:
                        il.pop(0)[1]()
                    if skc >= 2:
                        attnv(skc - 2)
                for t in il:
                    t[1]()
                attnv(SKC - 2)
                attnv(SKC - 1)
                return po

            def emit_normalize(j, p, po, snd):
                """oT chunks normalized: own slot (j==3) into ol, else into
                the send-staging tile snd[:, p, :]."""
                for ch in range(2):
                    rcp = wrk2.tile([128, 512], F32, tag="rcp", name="rcp")
                    rlo = wrk2.tile([64, 512], F32, tag="rlo", name="rlo")
                    nc.vector.reciprocal_approx_fast(out=rcp[:], in_=po[ch][:])
                    # partition-shift via scalar queue: sync is busy with
                    # x loads early on, scalar only has cheap dispatches
                    nc.scalar.dma_start(rlo[:], rcp[64:128, :])
                    ocp = wrk2.tile([64, 512], BF16, tag="ocp", name="ocp")
                    nc.vector.tensor_copy(ocp[:], po[ch][0:DK, :])
                    if j == 3:
                        dst = ol[p][64 * ch : 64 * (ch + 1), :]
                    else:
                        dst = snd[64 * ch : 64 * (ch + 1), p, :]
                    nc.vector.tensor_tensor(
                        dst, ocp[:], rlo[:], mybir.AluOpType.mult
                    )

            def emit_send(j, p, snd):
                # per-pair half so only pair 1's 128 KB remains at slot end
                nc.sync.dma_start(
                    cc_in[j][128 * p : 128 * (p + 1), :], snd[:, p, :]
                )

            def emit_ag(j):
                nc.gpsimd.collective_compute(
                    "AllGather",
                    mybir.AluOpType.bypass,
                    replica_groups=RG[j],
                    ins=[cc_in[j][:, :]],
                    outs=[cc_out[j][:, :]],
                )

            def emit_recv_sel(j):
                # AG out halves sum to own+peer; subtract the kept own send.
                # Plain tensor-tensor ops only (Pool can't do scalar-ptr ops).
                olr = recv.tile([128, 4, 512], BF16, tag="olr", name="olr")
                nc.gpsimd.dma_start(
                    olr[:],
                    cc_out[j].rearrange("(w p r) s -> r (w p) s", w=2, p=2),
                )
                for p in range(2):
                    tsel = recv.tile([128, 512], BF16, tag="tsel", name="tsel")
                    nc.gpsimd.tensor_add(tsel[:], olr[:, p, :], olr[:, 2 + p, :])
                    nc.gpsimd.tensor_sub(
                        ol[2 + 2 * j + p][:], tsel[:], snd_t[j][:, p, :]
                    )

            # ---- emission: prefix projections, then the 4-slot loop ----
            emit_q(0, 0)
            emit_k(0, 0)
            emit_k(1, 0)
            emit_v(range(0, 4))

            # projection work still owed after the prefix, as interleave
            # thunks: (consume-by iteration, thunk).  Spread across slot 0/1
            # units so attention starts early but nothing starves.
            def unit_interleave(j, p):
                if j == 0 and p == 0:
                    # v[k] must land before this unit's attnv(k) (iter k+2)
                    return [
                        (2, lambda: emit_v([4])),
                        (3, lambda: emit_v([5])),
                        (4, lambda: emit_k(2, 0)),
                        (5, lambda: emit_v([6])),
                        (6, lambda: emit_v([7])),
                        (7, lambda: emit_v([8])),
                        (8, lambda: emit_k(3, 0)),
                        (9, lambda: emit_v([9])),
                        (10, lambda: emit_v([10])),
                        (11, lambda: emit_v([11])),
                        (12, lambda: emit_v([12])),
                        (13, lambda: emit_v([13])),
                        (14, lambda: emit_v([14])),
                        (15, lambda: emit_v([15])),
                        (99, lambda: emit_q(0, 1)),
                        (99, lambda: emit_k(0, 1)),
                        (99, lambda: emit_k(1, 1)),
                    ]
                if j == 0 and p == 1:
                    return [
                        (4, lambda: emit_k(2, 1)),
                        (8, lambda: emit_k(3, 1)),
                        (12, lambda: emit_q(1, 0)),
                    ]
                if j == 1 and p == 0:
                    return [(8, lambda: emit_q(1, 1))]
                if j == 1 and p == 1:
                    return [(4, lambda: emit_q(2, 0)), (10, lambda: emit_q(2, 1))]
                if j == 2 and p == 0:
                    return [(8, lambda: emit_q(3, 0))]
                if j == 2 and p == 1:
                    return [(8, lambda: emit_q(3, 1))]
                return []

            for j in range(4):
                snd = snd_t[j] if j < 3 else None
                for p in range(2):
                    po = emit_attention(j, p, unit_interleave(j, p))
                    emit_normalize(j, p, po, snd)
                    if j < 3:
                        emit_send(j, p, snd)
                if j < 3:
                    emit_ag(j)
                # Pool-queue order: AG(0), AG(1), sel(0), AG(2), sel(2),
                # sel(1).  sel(1) must NOT precede AG(2): it waits on the
                # slow ring AG(1) and would head-of-line-block AG(2)'s
                # trigger.  sel(2) before sel(1) so the tail-critical mesh
                # chunks land first.
                if j == 1:
                    emit_recv_sel(0)
                elif j == 2:
                    emit_recv_sel(2)
            emit_recv_sel(1)

            if dbg:
                for p in range(2):
                    nc.sync.dma_start(d_qt[:, S * p : S * (p + 1)], qt_sb[p][:])
                    nc.sync.dma_start(d_kt[:, S * p : S * (p + 1)], kt_sb[p][:])
                for c8 in range(8):
                    nc.sync.dma_start(d_ol[:, 512 * c8 : 512 * (c8 + 1)], ol[c8][:])

            # ---- output projection: out[sq, :] = sum_c ol^T wo + bo ----
            # round-0 chunks first (land earliest); round-1 last — its ring
            # collective has the widest latency variance (22-83us observed)
            corder = [2, 3, 6, 7, 0, 1, 4, 5]
            for sb2 in range(SQ // 128):
                os_sb = wrk2.tile([128, DIN], F32, tag="os", name="os")
                pso = psmm.tile([128, 1024], F32, tag="mm", name="pso")
                for do in range(2):
                    for i, c in enumerate(corder):
                        nc.tensor.matmul(
                            pso[:, 512 * do : 512 * (do + 1)],
                            ol[c][:, 128 * sb2 : 128 * (sb2 + 1)],
                            wo_sb[:, c, 512 * do : 512 * (do + 1)],
                            start=(i == 0),
                            stop=(i == DC - 1),
                        )
                    nc.vector.tensor_add(
                        os_sb[:, 512 * do : 512 * (do + 1)],
                        pso[:, 512 * do : 512 * (do + 1)],
                        bo_sb[:, 512 * do : 512 * (do + 1)],
                    )
                    nc.sync.dma_start(
                        out[128 * sb2 : 128 * (sb2 + 1), 512 * do : 512 * (do + 1)],
                        os_sb[:, 512 * do : 512 * (do + 1)],
                    )

    nc.compile()
    return nc


_NC = None


def _get_nc():
    global _NC
    if _NC is None:
        _NC = build()
    return _NC


def _pack_x(Xb):
    """[2048, 1024] f32 -> [128, 4, DC, 512] bf16 s-block-major blocks."""
    xt = np.ascontiguousarray(Xb.T)  # [1024, 2048]
    # [c, p, sblk, s] -> [p, sblk, c, s]
    return xt.reshape(DC, 128, 4, 512).transpose(1, 2, 0, 3).astype(BF)


def _flat_x(x4):
    return np.ascontiguousarray(x4.reshape(128, 4 * DC * 512))


def _pack_w(W4):
    """[4, 1024, 64] -> [128, 8*256] bf16 ([part, c, pair-major cols])."""
    w = W4.transpose(1, 0, 2).reshape(DIN, HL * DK)  # col = 64*h_local + d
    w = w.reshape(DC, 128, HL * DK).transpose(1, 0, 2)
    return np.ascontiguousarray(w.reshape(128, DC * HL * DK)).astype(BF)


def _pack_wo(Wo, ell):
    """[1024, 1024] -> [128, 8*1024] bf16 for core with head-group ell.

    ol chunk c holds heads (4*g + 2*p + ch) for (g, p): chunks 0..1 own
    (g=ell, p=c), chunk 2+2j+p peer (g=ell^(j+1), p).  Within a chunk,
    partition row = 64*ch + d  <->  Wo row 64*head + d."""
    chunk_groups = [ell, ell]
    chunk_pairs = [0, 1]
    for j in range(3):
        for p in range(2):
            chunk_groups.append(ell ^ DELTAS[j])
            chunk_pairs.append(p)
    rows = np.empty((8, 128), np.int64)
    for c in range(8):
        g, p = chunk_groups[c], chunk_pairs[c]
        for ch in range(2):
            head = 4 * g + 2 * p + ch
            rows[c, 64 * ch : 64 * (ch + 1)] = 64 * head + np.arange(64)
    w = Wo[rows.reshape(-1), :]  # [8*128, 1024]
    w = w.reshape(8, 128, DIN).transpose(1, 0, 2)  # [128, 8, 1024]
    return np.ascontiguousarray(w.reshape(128, DC * DIN)).astype(BF)


def make_in_maps(Q, K, V, Wq, bq, Wk, bk, Wv, bv, Wo, bo):
    Q, K, V = (np.asarray(a, np.float32) for a in (Q, K, V))
    Wq, bq, Wk, bk, Wv, bv = (
        np.asarray(a, np.float32) for a in (Wq, bq, Wk, bk, Wv, bv)
    )
    Wo = np.asarray(Wo, np.float32)
    bo = np.asarray(bo, np.float32)
    xpk = [(_pack_x(Q[b]), _pack_x(K[b]), _pack_x(V[b])) for b in range(B)]
    bo_p = np.ascontiguousarray(np.broadcast_to(bo, (128, DIN)))
    in_maps = []
    for c in range(NCORES):
        b, ell = divmod(c, 4)
        hs = slice(HL * ell, HL * (ell + 1))
        # xq slots: [l^2, l^3, l^1, l] (round deltas, own block last)
        perm = [ell ^ d for d in DELTAS] + [ell]
        xq_rot = np.ascontiguousarray(xpk[b][0][:, perm, :, :])
        in_maps.append(
            {
                "xq": _flat_x(xq_rot),
                "xk": _flat_x(xpk[b][1]),
                "xv": _flat_x(xpk[b][2]),
                "wq": _pack_w(Wq[hs]),
                "wk": _pack_w(Wk[hs]),
                "wv": _pack_w(Wv[hs]),
                "wo": _pack_wo(Wo, ell),
                "bqp": np.ascontiguousarray(bq[hs].reshape(2, 128).T),
                "bkp": np.ascontiguousarray(bk[hs].reshape(2, 128).T),
                "bvr": np.ascontiguousarray(
                    np.broadcast_to(bv[hs].reshape(-1), (128, HL * DK))
                ),
                "bor": bo_p,
            }
        )
    return in_maps


def run(nc, in_maps, **kwargs):
    return bass_utils.run_bass_kernel_spmd(
        nc, in_maps, core_ids=list(range(NCORES)), **kwargs
    )


def kernel(Q, K, V, Wq, bq, Wk, bk, Wv, bv, Wo, bo):
    nc = _get_nc()
    in_maps = make_in_maps(Q, K, V, Wq, bq, Wk, bk, Wv, bv, Wo, bo)
    res = run(nc, in_maps)
    full = np.empty((B, S, DIN), np.float32)
    for c in range(NCORES):
        b, g = divmod(c, 4)
        full[b, SQ * g : SQ * (g + 1), :] = res.results[c]["out"]
    return full
